# revision 1
# baseline (speedup 1.0000x reference)
"""Multi-head attention (RoPE, causal) Trainium2 kernel, 8-way sharded.

Sharding: core c -> (batch b = c//2, head-group g = c%2 of 8 heads).
Each core computes its batch/head-group's attention output projected through
its W_proj row-slice; the host sums the two partial projections per batch and
adds b_proj.

Per-core device pipeline (all matmul inputs fp16, PSUM accumulation fp32):
  1. qT/kT = W'^T @ x^T feature-major [dims, tokens], with W' column-permuted
     on the host so each head's dims are [evens | odds] (RoPE de-interleave).
     RoPE applied via a 32-partition-block swap (SBUF->SBUF DMA) plus
     elementwise cos/sin tables.  v computed token-major [tokens, dims].
  2. Flash-style attention in S^T layout: S^T[k,q] tiles via row-packed
     (2 heads concurrent) K=64 matmuls; exp on ScalarE from PSUM with the
     1/sqrt(Dh) scale fused; causal mask = multiply by a 0/1 table on
     diagonal tiles only; P^T@V via col-packed (2 heads) matmuls; softmax
     denominators via col-packed ones-row matmuls; normalization deferred to
     a single reciprocal+broadcast multiply per (head-pair, q-chunk).
  3. out = outT^T @ W_proj_slice directly from the feature-major slab.
"""

import numpy as np

import concourse.bass as bass
import concourse.bacc as bacc
import concourse.tile as tile
import concourse.mybir as mybir
from concourse.bass_utils import run_bass_kernel_spmd

F16 = mybir.dt.float16
F32 = mybir.dt.float32

B, L, D = 4, 2048, 1024
H, Dh = 16, 64
ROPE_THETA = 10000.0
N_CORES = 8
HL = 8           # heads per core
DC = D // 128    # 8 contraction chunks
NJJ = 4          # head pairs per core
NTC = L // 512   # 4 token chunks of 512
NTT = L // 128   # 16 token tiles of 128
NQC = L // 512   # 4 q chunks of 512
NKT = L // 128   # 16 k tiles of 128


def _emit(nc, tc, dram, debug=False):
    """Emit the per-core Tile program."""
    from contextlib import ExitStack

    with ExitStack() as ctx:
        consts = ctx.enter_context(tc.tile_pool(name="consts", bufs=1))
        rope = ctx.enter_context(tc.tile_pool(name="rope", bufs=4))
        ptp = ctx.enter_context(tc.tile_pool(name="ptp", bufs=4))
        small = ctx.enter_context(tc.tile_pool(name="small", bufs=2))
        ostg = ctx.enter_context(tc.tile_pool(name="ostg", bufs=3))

        # ---- resident tensors -------------------------------------------
        xT = consts.tile([128, DC, L], F16)
        wq = consts.tile([128, DC, 512], F16)
        wk = consts.tile([128, DC, 512], F16)
        wv = consts.tile([128, DC, 512], F16)
        wp = consts.tile([128, NJJ, 1024], F16)
        cos4 = consts.tile([128, L], F16)
        sin4 = consts.tile([128, L], F16)
        masks = consts.tile([128, 4, 512], F16)
        bq = consts.tile([128, NJJ], F32)
        bk = consts.tile([128, NJJ], F32)
        bv = consts.tile([1, 512], F16)
        ones1 = consts.tile([1, 128], F16)
        qT = consts.tile([128, NJJ, L], F16)
        kT = consts.tile([128, NJJ, L], F16)
        vaug = consts.tile([128, NTT, HL, 128], F16)
        oT = consts.tile([128, NJJ, L], F16)

        nc.sync.dma_start(bq[:], dram["bq"].ap())
        for dc in range(DC):
            nc.sync.dma_start(wq[:, dc, :], dram["wq"].ap()[:, dc, :])
            nc.sync.dma_start(xT[:, dc, :], dram["xT"].ap()[:, dc, :])
        nc.sync.dma_start(bk[:], dram["bk"].ap())
        nc.sync.dma_start(wk[:], dram["wk"].ap())
        nc.sync.dma_start(cos4[:], dram["cos4"].ap())
        nc.sync.dma_start(sin4[:], dram["sin4"].ap())
        nc.sync.dma_start(wv[:], dram["wv"].ap())
        nc.sync.dma_start(bv[:], dram["bv"].ap())
        nc.sync.dma_start(masks[:], dram["masks"].ap())
        nc.sync.dma_start(wp[:], dram["wp"].ap())
        nc.vector.memset(ones1[:], 1.0)
        nc.gpsimd.memset(vaug[:, :, :, 64:128], 1.0)

        # ---- phase B: projections ---------------------------------------
        def rope_store(ps, b_sb, dstT, jj, ts):
            # RoPE: raw = ps + bias; swp = 32-block swap; out = raw*cos+swp*sin
            # head pairs 2/3 are emitted as fillers inside attention: keep
            # their psum-evacuation off ScalarE so exp never queues behind it
            raw = rope.tile([128, 512], F16, tag="raw", name=f"raw_{jj}_{ts}")
            nc.scalar.activation(
                raw[:], ps[:],
                mybir.ActivationFunctionType.Identity,
                bias=b_sb[:, jj:jj + 1],
            )
            swp = rope.tile([128, 512], F16, tag="swp", name=f"swp_{jj}_{ts}")
            for blk in range(4):
                sb_ = (blk ^ 1) * 32
                nc.sync.dma_start(
                    swp[blk * 32:(blk + 1) * 32, :], raw[sb_:sb_ + 32, :]
                )
            t1 = rope.tile([128, 512], F16, tag="t1", name=f"t1_{jj}_{ts}")
            nc.vector.tensor_mul(t1[:], raw[:], cos4[:, ts:ts + 512])
            t2 = rope.tile([128, 512], F16, tag="t2", name=f"t2_{jj}_{ts}")
            nc.vector.tensor_mul(t2[:], swp[:], sin4[:, ts:ts + 512])
            nc.gpsimd.tensor_add(dstT[:, jj, ts:ts + 512], t1[:], t2[:])

        def qk_gemm_dcouter(jj, pg):
            # dc-outer: consume xT chunk-by-chunk as it streams in
            for w_sb, b_sb, dstT in ((wq, bq, qT), (wk, bk, kT)):
                pss = [pg.tile([128, 512], F32, tag=f"g{tcn}",
                               name=f"g{tcn}_{0 if dstT is qT else 1}")
                       for tcn in range(NTC)]
                for dc in range(DC):
                    for tcn in range(NTC):
                        nc.tensor.matmul(
                            pss[tcn][:],
                            w_sb[:, dc, jj * 128:(jj + 1) * 128],
                            xT[:, dc, tcn * 512:tcn * 512 + 512],
                            start=(dc == 0),
                            stop=(dc == DC - 1),
                        )
                for tcn in range(NTC):
                    rope_store(pss[tcn], b_sb, dstT, jj, tcn * 512)

        def qk_gemm(jj, pg):
            for w_sb, b_sb, dstT in ((wq, bq, qT), (wk, bk, kT)):
                for tcn in range(NTC):
                    ts = tcn * 512
                    ps = pg.tile([128, 512], F32, tag="gemm")
                    for dc in range(DC):
                        nc.tensor.matmul(
                            ps[:],
                            w_sb[:, dc, jj * 128:(jj + 1) * 128],
                            xT[:, dc, ts:ts + 512],
                            start=(dc == 0),
                            stop=(dc == DC - 1),
                        )
                    rope_store(ps, b_sb, dstT, jj, ts)

        def v_gemm(pg):
            for tt in range(NTT):
                ps = pg.tile([128, 512], F32, tag="gemm")
                for dc in range(DC):
                    nc.tensor.matmul(
                        ps[:],
                        xT[:, dc, tt * 128:(tt + 1) * 128],
                        wv[:, dc, :],
                        start=(dc == 0),
                        stop=False,
                    )
                nc.tensor.matmul(
                    ps[:], ones1[:], bv[:], start=False, stop=True,
                )
                nc.scalar.copy(
                    vaug[:, tt, :, 0:64],
                    ps[:].rearrange("p (h d) -> p h d", h=HL),
                )

        # ---- phase C: attention for one (head pair, q chunk) ------------
        def attention_block(jj, qc, pp, po):
            qs = qc * 512
            pso = [po.tile([128, 512], F32, tag="psoA", name=f"psoA_{jj}_{qc}"),
                   po.tile([128, 512], F32, tag="psoB", name=f"psoB_{jj}_{qc}")]
            nkt = 4 * qc + 4
            for kt in range(nkt):
                ks = kt * 128
                pst = pp.tile([128, 2, 512], F32, tag="pst")
                nc.tensor.matmul(
                    pst[:, 0, :],
                    kT[0:64, jj, ks:ks + 128],
                    qT[0:64, jj, qs:qs + 512],
                    start=True, stop=True,
                )
                nc.tensor.matmul(
                    pst[:, 1, :],
                    kT[64:128, jj, ks:ks + 128],
                    qT[64:128, jj, qs:qs + 512],
                    start=True, stop=True,
                )
                diag = kt - 4 * qc
                ptile = ptp.tile([128, 2, 512], F16, tag="ptile")
                if diag > 0:
                    c0 = 128 * diag
                    tmp = ptp.tile([128, 2, 512], F16, tag="tmp")
                    nc.vector.memset(ptile[:, :, 0:c0], 0.0)
                    nc.scalar.activation(
                        tmp[:, :, c0:512], pst[:, :, c0:512],
                        mybir.ActivationFunctionType.Exp, scale=0.125,
                    )
                    for h in range(2):
                        nc.vector.tensor_mul(
                            ptile[:, h, c0:512], tmp[:, h, c0:512],
                            masks[:, diag, c0:512],
                        )
                elif diag == 0:
                    tmp = ptp.tile([128, 2, 512], F16, tag="tmp")
                    nc.scalar.activation(
                        tmp[:], pst[:],
                        mybir.ActivationFunctionType.Exp, scale=0.125,
                    )
                    for h in range(2):
                        nc.vector.tensor_mul(
                            ptile[:, h, :], tmp[:, h, :], masks[:, 0, :]
                        )
                else:
                    nc.scalar.activation(
                        ptile[:], pst[:],
                        mybir.ActivationFunctionType.Exp, scale=0.125,
                    )
                first, last = kt == 0, kt == nkt - 1
                for h in range(2):
                    nc.tensor.matmul(
                        pso[h][:], vaug[:, kt, 2 * jj + h, :],
                        ptile[:, h, :],
                        start=first, stop=last,
                    )
            # normalize: oT rows = psoX[0:64] * recip(den rows 64:128)
            for h in range(2):
                den_sb = small.tile([64, 512], F32, tag="densb")
                nc.vector.tensor_copy(den_sb[:], pso[h][64:128, :])
                rec = small.tile([64, 512], F32, tag="rec")
                nc.vector.reciprocal_approx_fast(rec[:], den_sb[:])
                nc.vector.tensor_mul(
                    oT[64 * h:64 * h + 64, jj, qs:qs + 512],
                    pso[h][0:64, :], rec[:],
                )

        # ---- phase D: output projection (per q-chunk) -------------------
        def proj_block(qc, pg):
            for tt in range(4 * qc, 4 * qc + 4):
                for cc in range(2):
                    ps = pg.tile([128, 512], F32, tag="gemm")
                    for jj in range(NJJ):
                        nc.tensor.matmul(
                            ps[:],
                            oT[:, jj, tt * 128:(tt + 1) * 128],
                            wp[:, jj, cc * 512:cc * 512 + 512],
                            start=(jj == 0), stop=(jj == NJJ - 1),
                        )
                    stage = ostg.tile([128, 512], F32)
                    nc.vector.tensor_copy(stage[:], ps[:])
                    nc.sync.dma_start(
                        dram["out"].ap()[tt * 128:(tt + 1) * 128,
                                         cc * 512:cc * 512 + 512],
                        stage[:],
                    )

        with (
            tc.tile_pool(name="pgS1", bufs=1, space="PSUM") as pgs1,
            tc.tile_pool(name="pgS2", bufs=3, space="PSUM") as pgs2,
        ):
            qk_gemm_dcouter(0, pgs1)
            v_gemm(pgs2)
        with (
            tc.tile_pool(name="pgB", bufs=2, space="PSUM") as pgb,
            tc.tile_pool(name="pp", bufs=2, space="PSUM") as pp,
            tc.tile_pool(name="po", bufs=1, space="PSUM") as po,
        ):
            for qc in range(NQC):
                attention_block(0, qc, pp, po)
                if qc == 0:
                    qk_gemm(1, pgb)
            for qc in range(NQC):
                attention_block(1, qc, pp, po)
                if qc == 0:
                    qk_gemm(2, pgb)
            for qc in range(NQC):
                attention_block(2, qc, pp, po)
                if qc == 1:
                    qk_gemm(3, pgb)
            for qc in range(NQC):
                attention_block(3, qc, pp, po)
                proj_block(qc, pgb)

        if debug:
            nc.sync.dma_start(dram["dbg_qT"].ap(), qT[:])
            nc.sync.dma_start(dram["dbg_kT"].ap(), kT[:])
            nc.sync.dma_start(dram["dbg_vaug"].ap(), vaug[:])
            nc.sync.dma_start(dram["dbg_oT"].ap(), oT[:])


def build(debug=False):
    nc = bacc.Bacc("TRN2", target_bir_lowering=False, debug=False)
    dram = {
        "xT": nc.dram_tensor("xT", [128, DC, L], F16, kind="ExternalInput"),
        "wq": nc.dram_tensor("wq", [128, DC, 512], F16, kind="ExternalInput"),
        "wk": nc.dram_tensor("wk", [128, DC, 512], F16, kind="ExternalInput"),
        "wv": nc.dram_tensor("wv", [128, DC, 512], F16, kind="ExternalInput"),
        "wp": nc.dram_tensor("wp", [128, NJJ, 1024], F16, kind="ExternalInput"),
        "cos4": nc.dram_tensor("cos4", [128, L], F16, kind="ExternalInput"),
        "sin4": nc.dram_tensor("sin4", [128, L], F16, kind="ExternalInput"),
        "masks": nc.dram_tensor("masks", [128, 4, 512], F16, kind="ExternalInput"),
        "bq": nc.dram_tensor("bq", [128, NJJ], F32, kind="ExternalInput"),
        "bk": nc.dram_tensor("bk", [128, NJJ], F32, kind="ExternalInput"),
        "bv": nc.dram_tensor("bv", [1, 512], F16, kind="ExternalInput"),
        "out": nc.dram_tensor("out", [L, D], F32, kind="ExternalOutput"),
    }
    if debug:
        dram["dbg_qT"] = nc.dram_tensor("dbg_qT", [128, NJJ, L], F16, kind="ExternalOutput")
        dram["dbg_kT"] = nc.dram_tensor("dbg_kT", [128, NJJ, L], F16, kind="ExternalOutput")
        dram["dbg_vaug"] = nc.dram_tensor("dbg_vaug", [128, NTT, HL, 128], F16, kind="ExternalOutput")
        dram["dbg_oT"] = nc.dram_tensor("dbg_oT", [128, NJJ, L], F16, kind="ExternalOutput")
    with tile.TileContext(nc) as tc:
        _emit(nc, tc, dram, debug=debug)
    nc.compile()
    return nc


def host_inputs(x, W_qkv, b_qkv, W_proj):
    """Build the 8 per-core input maps (numpy, fp16-cast, pre-laid-out)."""
    x = np.asarray(x, np.float32)
    W_qkv = np.asarray(W_qkv, np.float32)
    b_qkv = np.asarray(b_qkv, np.float32)
    W_proj = np.asarray(W_proj, np.float32)

    # RoPE tables in the de-interleaved (evens|odds) per-32-block layout.
    inv_freq = 1.0 / (ROPE_THETA ** (np.arange(0, Dh, 2, dtype=np.float32) / Dh))
    t = np.arange(L, dtype=np.float32)
    freqs = np.outer(t, inv_freq)            # [L, 32]
    cosT = np.cos(freqs).T.astype(np.float32)  # [32, L]
    sinT = np.sin(freqs).T.astype(np.float32)
    cos4 = np.tile(cosT, (4, 1)).astype(np.float16)           # [128, L]
    sin4 = np.concatenate([-sinT, sinT, -sinT, sinT], 0).astype(np.float16)

    # causal masks for the 4 diagonal offsets: keep iff q >= k + 128*i
    kk = np.arange(128)[:, None]
    qq = np.arange(512)[None, :]
    masks = np.stack(
        [(qq >= kk + 128 * i) for i in range(4)], axis=1
    ).astype(np.float16)                     # [128, 4, 512]

    perm = np.concatenate([np.arange(0, Dh, 2), np.arange(1, Dh, 2)])  # evens|odds

    in_maps = []
    for c in range(N_CORES):
        b, g = c // 2, c % 2
        heads = np.arange(g * HL, g * HL + HL)
        qk_cols = np.concatenate([h * Dh + perm for h in heads])       # [512]
        v_lo = 2 * D + g * 512

        xT = np.ascontiguousarray(x[b].T)                  # [D, L]
        xT = xT.reshape(DC, 128, L).transpose(1, 0, 2)     # [128, DC, L]

        def wslice(cols_base, cols):
            w = W_qkv[:, cols_base + cols] if cols is not None \
                else W_qkv[:, cols_base:cols_base + 512]
            return np.ascontiguousarray(
                w.reshape(DC, 128, 512).transpose(1, 0, 2)).astype(np.float16)

        wq_h = wslice(0, qk_cols)
        wk_h = wslice(D, qk_cols)
        wv_h = wslice(v_lo, None)
        wp_h = np.ascontiguousarray(
            W_proj[g * 512:(g + 1) * 512, :]
            .reshape(NJJ, 128, 1024).transpose(1, 0, 2)).astype(np.float16)
        bq_h = np.ascontiguousarray(
            b_qkv[qk_cols].reshape(NJJ, 128).T).astype(np.float32)
        bk_h = np.ascontiguousarray(
            b_qkv[D + qk_cols].reshape(NJJ, 128).T).astype(np.float32)
        bv_h = b_qkv[v_lo:v_lo + 512].reshape(1, 512).astype(np.float16)

        in_maps.append({
            "xT": xT.astype(np.float16),
            "wq": wq_h, "wk": wk_h, "wv": wv_h, "wp": wp_h,
            "cos4": cos4, "sin4": sin4, "masks": masks,
            "bq": bq_h, "bk": bk_h, "bv": bv_h,
        })
    return in_maps


_NC = None


def kernel(x, W_qkv, b_qkv, W_proj, b_proj, attention_mask):
    global _NC
    if _NC is None:
        _NC = build()
    in_maps = host_inputs(x, W_qkv, b_qkv, W_proj)
    res = run_bass_kernel_spmd(_NC, in_maps, core_ids=list(range(N_CORES)))
    b_proj = np.asarray(b_proj, np.float32)
    out = np.empty((B, L, D), np.float32)
    for b in range(B):
        out[b] = res.results[2 * b]["out"] + res.results[2 * b + 1]["out"] + b_proj
    return out

